# revision 4
# baseline (speedup 1.0000x reference)
"""Trainium2 Bass kernel for nn_CategoryAdder (embedding lookup + masked add).

Computation: out[b,s,:] = inputs[b,s,:] + emb where
  emb = table[categories[b,s]] masked to zero when categories[b,s]==0 or
  s == mask_positions[b].

Host-side preprocessing folds both masks into the data:
  - categories[b, mask_positions[b]] = 0
  - table row 0 zeroed (on a copy)
so the device computes exactly: out = inputs + table0[categories].

All data streams are bf16 (converted host-side): the baseline fp32 kernel was
DMA-engine-bus bound (16 engines x 22.5 GB/s, each ~93% busy moving 96 MiB per
core), so halving bytes halves the floor. bf16 round-trip error is ~0.3%
against the 2e-2 rel-err gate.

Sharding: data-parallel over batch across 8 NeuronCores (8 batches per core,
16384 tokens/core). Table replicated. Per core the kernel loops over tiles of
T tokens: SWDGE dma_gather pulls the 1 KB bf16 table rows from HBM by
precomputed int16 indices, HWDGE loads the input tile, DVE adds in bf16 (2x
throughput), HWDGE stores bf16. Host converts the output back to fp32. Tile
sizes are graduated (small first/last) so the pipeline fills and drains faster.
"""

import numpy as np
import ml_dtypes

import concourse.mybir as mybir
from concourse import bacc, tile
from concourse.bass_utils import run_bass_kernel_spmd

BF16 = ml_dtypes.bfloat16


def _ensure_axon_ntff_hook_module():
    """run_bass_kernel_spmd(trace=True) under axon imports antenv.axon_hooks,
    which this image lacks — install a fallback shim (backed by the boot
    module's ctypes hook when available) so a BASS_TRACE=1 environment does
    not crash the kernel. No-op when the real module exists."""
    try:
        import antenv.axon_hooks  # noqa: F401
        return
    except ImportError:
        pass
    import sys
    import types

    hook = None
    try:
        import trn_agent_boot.trn_boot as _tb

        hook = _tb._ntff_profile_via_ctypes("/opt/axon/libaxon_pjrt.so")
    except Exception:
        hook = None  # get_..._hook() -> None makes bass_utils skip tracing
    mod = types.ModuleType("antenv.axon_hooks")
    mod.get_axon_ntff_profile_hook = lambda: hook
    mod.set_axon_ntff_profile_hook = lambda h: None
    sys.modules["antenv.axon_hooks"] = mod


_ensure_axon_ntff_hook_module()

B, S, D = 64, 2048, 512
N_CAT = 5000
N_CORES = 8
B_PER = B // N_CORES          # 8 batches per core
NTOK = B_PER * S              # 16384 tokens per core
IDX_COLS = NTOK // 16         # columns of the wrapped int16 index tensor

# Tile schedule (tokens per tile): small tiles prime the pipeline at the start
# and shorten the serial add+store chain at the end; 2048-token middles halve
# the per-instruction Q7 fixed overhead (~1us each) on the critical gen path.
TILES = [256, 256, 512, 1024] + [2048] * 6 + [1024] + [256] * 4
assert sum(TILES) == NTOK
N_HEAD = 3  # tiles whose indices ride the small head idx DMA


def _build_nc():
    nc = bacc.Bacc("TRN2", target_bir_lowering=False, debug=False)
    x = nc.dram_tensor("x", [NTOK, D], mybir.dt.bfloat16, kind="ExternalInput")
    tbl = nc.dram_tensor("tbl", [N_CAT, D], mybir.dt.bfloat16, kind="ExternalInput")
    idx = nc.dram_tensor("idx", [128, IDX_COLS], mybir.dt.int16, kind="ExternalInput")
    out = nc.dram_tensor("out", [NTOK, D], mybir.dt.bfloat16, kind="ExternalOutput")

    head = sum(t // 16 for t in TILES[:N_HEAD])
    with tile.TileContext(nc) as tc:
        with (
            tc.tile_pool(name="idxp", bufs=1) as idxp,
            tc.tile_pool(name="inp", bufs=4) as inp,
            tc.tile_pool(name="embp", bufs=3) as embp,
        ):
            # Two separate idx tiles (separate semaphores): the first gather
            # only waits on the 16KB head DMA, not the full idx transfer.
            idx_head = idxp.tile([128, head], mybir.dt.int16, tag="idxh")
            idx_tail = idxp.tile([128, IDX_COLS - head], mybir.dt.int16, tag="idxt")
            nc.sync.dma_start(out=idx_head[:], in_=idx[:, :head])
            nc.sync.dma_start(out=idx_tail[:], in_=idx[:, head:])
            t0 = 0
            col = 0
            for ti, T in enumerate(TILES):
                C = T // 128
                if ti < N_HEAD:
                    idx_ap = idx_head[:, col : col + T // 16]
                else:
                    idx_ap = idx_tail[:, col - head : col - head + T // 16]
                emb_t = embp.tile([128, C * D], mybir.dt.bfloat16, tag="emb")
                nc.gpsimd.dma_gather(
                    emb_t[:].rearrange("p (c e) -> p c e", e=D),
                    tbl[:, :],
                    idx_ap,
                    T,
                    T,
                    D,
                    # multi-packet lets the SDMA engines start draining while
                    # Q7 is still generating descriptors (~7ns/desc + 1us);
                    # single_packet also hard-fails above 1024 idxs.
                    single_packet=False,
                )
                in_t = inp.tile([128, C * D], mybir.dt.bfloat16, tag="in")
                nc.sync.dma_start(
                    out=in_t[:],
                    in_=x[t0 : t0 + T].rearrange("(p c) e -> p (c e)", p=128),
                )
                nc.vector.tensor_add(out=in_t[:], in0=in_t[:], in1=emb_t[:])
                nc.sync.dma_start(
                    out=out[t0 : t0 + T].rearrange("(p c) e -> p (c e)", p=128),
                    in_=in_t[:],
                )
                t0 += T
                col += T // 16
    nc.compile()
    return nc


def _prep_idx(cat_shard: np.ndarray) -> np.ndarray:
    """cat_shard: (NTOK,) int -> wrapped int16 index tensor [128, IDX_COLS].

    dma_gather writes gather-slot i to SBUF (partition i%128, column i//128);
    our tiles place token t at (partition t//C, column t%C), so slot i holds
    the category of token (i%128)*C + i//128. Indices are then wrapped 16-way
    (idxs[p, s] = slot s*16+p) and replicated across the 8 groups of 16
    partitions as the HW expects.
    """
    blocks = []
    t0 = 0
    for T in TILES:
        C = T // 128
        slot_to_token = (np.arange(T) % 128) * C + (np.arange(T) // 128)
        vals = cat_shard[t0 : t0 + T][slot_to_token]
        blocks.append(np.tile(vals.reshape(T // 16, 16).T, (8, 1)))
        t0 += T
    return np.ascontiguousarray(np.concatenate(blocks, axis=1).astype(np.int16))


RUN_KWARGS = {}  # test harness can set e.g. {"trace": True}
LAST_RESULTS = None
_NC = None


def _get_nc():
    global _NC
    if _NC is None:
        _NC = _build_nc()
    return _NC


def kernel(inputs, categories, mask_positions, table):
    global LAST_RESULTS
    inputs = np.asarray(inputs, dtype=np.float32)
    categories = np.asarray(categories).astype(np.int64)
    mask_positions = np.asarray(mask_positions).astype(np.int64)
    table = np.asarray(table, dtype=np.float32)

    # Fold both masks into the data.
    cat = categories.copy()
    cat[np.arange(B), mask_positions[:, 0]] = 0
    tbl0 = table.astype(BF16)
    tbl0[0] = 0.0

    nc = _get_nc()

    x_bf = inputs.reshape(B, S * D).astype(BF16)
    in_maps = []
    for c in range(N_CORES):
        x_shard = np.ascontiguousarray(
            x_bf[c * B_PER : (c + 1) * B_PER].reshape(NTOK, D)
        )
        cat_shard = cat[c * B_PER : (c + 1) * B_PER].reshape(NTOK)
        in_maps.append({"x": x_shard, "tbl": tbl0, "idx": _prep_idx(cat_shard)})

    res = run_bass_kernel_spmd(
        nc, in_maps, core_ids=list(range(N_CORES)), **RUN_KWARGS
    )
    LAST_RESULTS = res
    out = np.concatenate(
        [r["out"].astype(np.float32).reshape(B_PER, S, D) for r in res.results],
        axis=0,
    )
    return out


# revision 6
# speedup vs baseline: 1.0766x; 1.0766x over previous
"""Trainium2 Bass kernel for nn_CategoryAdder (embedding lookup + masked add).

Computation: out[b,s,:] = inputs[b,s,:] + emb where
  emb = table[categories[b,s]] masked to zero when categories[b,s]==0 or
  s == mask_positions[b].

Host-side preprocessing folds both masks into the data:
  - categories[b, mask_positions[b]] = 0
  - table row 0 zeroed (on a copy)
so the device computes exactly: out = inputs + table0[categories].

All data streams are bf16 (converted host-side): the baseline fp32 kernel was
DMA-engine-bus bound (16 engines x 22.5 GB/s, each ~93% busy moving 96 MiB per
core), so halving bytes halves the floor. bf16 round-trip error is ~0.3%
against the 2e-2 rel-err gate.

Sharding: data-parallel over batch across 8 NeuronCores (8 batches per core,
16384 tokens/core). Table replicated. Per core the kernel loops over tiles of
T tokens: SWDGE dma_gather pulls the 1 KB bf16 table rows from HBM by
precomputed int16 indices, HWDGE loads the input tile, DVE adds in bf16 (2x
throughput), HWDGE stores bf16. Host converts the output back to fp32. Tile
sizes are graduated (small first/last) so the pipeline fills and drains faster.
"""

import numpy as np
import ml_dtypes

import concourse.mybir as mybir
from concourse import bacc, tile
from concourse.bass_utils import run_bass_kernel_spmd

BF16 = ml_dtypes.bfloat16


def _ensure_axon_ntff_hook_module():
    """run_bass_kernel_spmd(trace=True) under axon imports antenv.axon_hooks,
    which this image lacks — install a fallback shim (backed by the boot
    module's ctypes hook when available) so a BASS_TRACE=1 environment does
    not crash the kernel. No-op when the real module exists."""
    try:
        import antenv.axon_hooks  # noqa: F401
        return
    except ImportError:
        pass
    import sys
    import types

    hook = None
    try:
        import trn_agent_boot.trn_boot as _tb

        hook = _tb._ntff_profile_via_ctypes("/opt/axon/libaxon_pjrt.so")
    except Exception:
        hook = None  # get_..._hook() -> None makes bass_utils skip tracing
    mod = types.ModuleType("antenv.axon_hooks")
    mod.get_axon_ntff_profile_hook = lambda: hook
    mod.set_axon_ntff_profile_hook = lambda h: None
    sys.modules["antenv.axon_hooks"] = mod


_ensure_axon_ntff_hook_module()

B, S, D = 64, 2048, 512
N_CAT = 5000
N_CORES = 8
B_PER = B // N_CORES          # 8 batches per core
NTOK = B_PER * S              # 16384 tokens per core
IDX_COLS = NTOK // 16         # columns of the wrapped int16 index tensor

# Tile schedule (tokens per tile): small tiles prime the pipeline at the start
# and shorten the serial add+store chain at the end; 2048-token middles halve
# the per-instruction Q7 fixed overhead (~1us each) on the critical gen path.
TILES = [256, 256, 512, 1024] + [2048] * 6 + [1024, 512, 512]
assert sum(TILES) == NTOK
N_HEAD = 3  # tiles whose indices ride the small head idx DMA


def _build_nc():
    nc = bacc.Bacc("TRN2", target_bir_lowering=False, debug=False)
    x = nc.dram_tensor("x", [NTOK, D], mybir.dt.bfloat16, kind="ExternalInput")
    tbl = nc.dram_tensor("tbl", [N_CAT, D], mybir.dt.bfloat16, kind="ExternalInput")
    idx = nc.dram_tensor("idx", [128, IDX_COLS], mybir.dt.int16, kind="ExternalInput")
    out = nc.dram_tensor("out", [NTOK, D], mybir.dt.bfloat16, kind="ExternalOutput")

    head = sum(t // 16 for t in TILES[:N_HEAD])
    with tile.TileContext(nc) as tc:
        with (
            tc.tile_pool(name="idxp", bufs=1) as idxp,
            tc.tile_pool(name="inp", bufs=5) as inp,
            tc.tile_pool(name="embp", bufs=5) as embp,
        ):
            # Two separate idx tiles (separate semaphores): the first gather
            # only waits on the 16KB head DMA, not the full idx transfer.
            idx_head = idxp.tile([128, head], mybir.dt.int16, tag="idxh")
            idx_tail = idxp.tile([128, IDX_COLS - head], mybir.dt.int16, tag="idxt")
            nc.sync.dma_start(out=idx_head[:], in_=idx[:, :head])
            nc.sync.dma_start(out=idx_tail[:], in_=idx[:, head:])
            t0 = 0
            col = 0
            for ti, T in enumerate(TILES):
                C = T // 128
                if ti < N_HEAD:
                    idx_ap = idx_head[:, col : col + T // 16]
                else:
                    idx_ap = idx_tail[:, col - head : col - head + T // 16]
                emb_t = embp.tile([128, C * D], mybir.dt.bfloat16, tag="emb")
                nc.gpsimd.dma_gather(
                    emb_t[:].rearrange("p (c e) -> p c e", e=D),
                    tbl[:, :],
                    idx_ap,
                    T,
                    T,
                    D,
                    # multi-packet lets the SDMA engines start draining while
                    # Q7 is still generating descriptors (~7ns/desc + 1us);
                    # single_packet also hard-fails above 1024 idxs.
                    single_packet=False,
                )
                in_t = inp.tile([128, C * D], mybir.dt.bfloat16, tag="in")
                nc.sync.dma_start(
                    out=in_t[:],
                    in_=x[t0 : t0 + T].rearrange("(p c) e -> p (c e)", p=128),
                )
                nc.vector.tensor_add(out=in_t[:], in0=in_t[:], in1=emb_t[:])
                nc.sync.dma_start(
                    out=out[t0 : t0 + T].rearrange("(p c) e -> p (c e)", p=128),
                    in_=in_t[:],
                )
                t0 += T
                col += T // 16
    nc.compile()
    return nc


def _prep_idx(cat_shard: np.ndarray) -> np.ndarray:
    """cat_shard: (NTOK,) int -> wrapped int16 index tensor [128, IDX_COLS].

    dma_gather writes gather-slot i to SBUF (partition i%128, column i//128);
    our tiles place token t at (partition t//C, column t%C), so slot i holds
    the category of token (i%128)*C + i//128. Indices are then wrapped 16-way
    (idxs[p, s] = slot s*16+p) and replicated across the 8 groups of 16
    partitions as the HW expects.
    """
    blocks = []
    t0 = 0
    for T in TILES:
        C = T // 128
        slot_to_token = (np.arange(T) % 128) * C + (np.arange(T) // 128)
        vals = cat_shard[t0 : t0 + T][slot_to_token]
        blocks.append(np.tile(vals.reshape(T // 16, 16).T, (8, 1)))
        t0 += T
    return np.ascontiguousarray(np.concatenate(blocks, axis=1).astype(np.int16))


RUN_KWARGS = {}  # test harness can set e.g. {"trace": True}
LAST_RESULTS = None
_NC = None


def _get_nc():
    global _NC
    if _NC is None:
        _NC = _build_nc()
    return _NC


def kernel(inputs, categories, mask_positions, table):
    global LAST_RESULTS
    inputs = np.asarray(inputs, dtype=np.float32)
    categories = np.asarray(categories).astype(np.int64)
    mask_positions = np.asarray(mask_positions).astype(np.int64)
    table = np.asarray(table, dtype=np.float32)

    # Fold both masks into the data.
    cat = categories.copy()
    cat[np.arange(B), mask_positions[:, 0]] = 0
    tbl0 = table.astype(BF16)
    tbl0[0] = 0.0

    nc = _get_nc()

    x_bf = inputs.reshape(B, S * D).astype(BF16)
    in_maps = []
    for c in range(N_CORES):
        x_shard = np.ascontiguousarray(
            x_bf[c * B_PER : (c + 1) * B_PER].reshape(NTOK, D)
        )
        cat_shard = cat[c * B_PER : (c + 1) * B_PER].reshape(NTOK)
        in_maps.append({"x": x_shard, "tbl": tbl0, "idx": _prep_idx(cat_shard)})

    res = run_bass_kernel_spmd(
        nc, in_maps, core_ids=list(range(N_CORES)), **RUN_KWARGS
    )
    LAST_RESULTS = res
    out = np.concatenate(
        [r["out"].astype(np.float32).reshape(B_PER, S, D) for r in res.results],
        axis=0,
    )
    return out
